# revision 1
# baseline (speedup 1.0000x reference)
"""Trainium2 Bass kernel for a quantized BertSelfOutput block.

Computation (per batch element, data-parallel over 8 NeuronCores):
    xq = clip(round(x / act_scale), -128, 127)            (kept as integers)
    qw = clip(round(W / w_scale[o]), -128, 127)           (kept as integers)
    y[t,o] = (sum_h xq[t,h]*qw[o,h]) * act_scale*w_scale[o] + b[o]
    h = bf16(y) + bf16(r)
    out = (h - mean_h) * rsqrt(var_h + eps) * gamma + beta

The integer quantized values (|q| <= 128) are exactly representable in
bf16 and the worst-case accumulated dot product (1024*128*127 < 2^24)
fits in fp32, so the bf16 TensorEngine matmul is numerically exact.

Rounding uses the fp32 magic-number trick: fp32(v + 1.5*2^23) rounds v
to the nearest integer (ties to even, matching jnp.round).
"""

import functools
import sys

sys.path.insert(0, "/opt/trn_rl_repo")

import numpy as np

import concourse.bass as bass
import concourse.mybir as mybir
import concourse.tile as tile
from concourse import bacc
from concourse.bass_utils import run_bass_kernel_spmd

dt = mybir.dt
Alu = mybir.AluOpType
Act = mybir.ActivationFunctionType

B, S, H = 8, 2048, 1024
P = 128
KT = H // P      # contraction tiles (8)
MT = S // P      # token tiles per core (16)
NB = H // 512    # psum bank halves of the output row (2)
MAGIC = 12582912.0   # 1.5 * 2**23
LN_EPS = 1e-12


def _build(apply_gamma: bool, apply_beta: bool, loop_reps: int = 0):
    """Build + compile the per-core program.

    loop_reps=0 -> plain single-pass kernel (graded path).
    loop_reps=R -> whole body wrapped in a dynamic For_i loop running R
                   times (benchmark builds; amortizes host/dispatch cost).
    """
    nc = bacc.Bacc("TRN2", target_bir_lowering=False, debug=False)

    x_d = nc.declare_dram_parameter("x", [S, H], dt.float32, False)
    r_d = nc.declare_dram_parameter("r", [S, H], dt.float32, False)
    w_d = nc.declare_dram_parameter("w", [H, H], dt.float32, False)
    scale_d = nc.declare_dram_parameter("scale_vec", [H], dt.float32, False)
    bias_d = nc.declare_dram_parameter("bias_vec", [H], dt.float32, False)
    inva_d = nc.declare_dram_parameter("inv_act", [P, 1], dt.float32, False)
    invw_d = nc.declare_dram_parameter("inv_w", [P, KT], dt.float32, False)
    if apply_gamma:
        gamma_d = nc.declare_dram_parameter("gamma_vec", [H], dt.float32, False)
    if apply_beta:
        beta_d = nc.declare_dram_parameter("beta_vec", [H], dt.float32, False)
    out_d = nc.declare_dram_parameter("out", [S, H], dt.float32, True)

    def bcast_load(handle):
        """DMA a [H] dram vector replicated across all 128 partitions."""
        t = singles.tile([P, H], dt.float32, tag=f"bc_{handle.name}")
        ap = handle[:]
        bc = bass.AP(tensor=ap.tensor, offset=ap.offset, ap=[[0, P], *ap.ap])
        nc.gpsimd.dma_start(out=t, in_=bc)
        return t

    with tile.TileContext(nc) as tc:
        with (
            tc.tile_pool(name="singles", bufs=1) as singles,
            tc.tile_pool(name="wstage", bufs=2) as wstage,
            tc.tile_pool(name="xstage", bufs=3) as xstage,
            tc.tile_pool(name="qstage", bufs=3) as qstage,
            tc.tile_pool(name="qtstage", bufs=3) as qtstage,
            tc.tile_pool(name="rstage", bufs=3) as rstage,
            tc.tile_pool(name="estage", bufs=3) as estage,
            tc.tile_pool(name="ostage", bufs=3) as ostage,
            tc.tile_pool(name="vecs", bufs=4) as vecs,
            tc.tile_pool(name="psum", bufs=2, space=bass.MemorySpace.PSUM) as psum,
        ):
            # ---- constants / broadcasts (outside any timing loop) ----
            scale_full = bcast_load(scale_d)
            bias_full = bcast_load(bias_d)
            gamma_full = bcast_load(gamma_d) if apply_gamma else None
            beta_full = bcast_load(beta_d) if apply_beta else None
            inva_sb = singles.tile([P, 1], dt.float32)
            nc.sync.dma_start(out=inva_sb, in_=inva_d[:])
            invw_sb = singles.tile([P, KT], dt.float32)
            nc.sync.dma_start(out=invw_sb, in_=invw_d[:])
            eps_sb = singles.tile([P, 1], dt.float32)
            nc.vector.memset(eps_sb, LN_EPS)

            WqT = singles.tile([P, KT, H], dt.bfloat16)  # [h_inner, h_tile, o]

            def body(_iv=None):
                # ---- quantize + transpose W ----
                for i in range(KT):  # o-tiles of W
                    wt = wstage.tile([P, H], dt.float32, tag="wt")
                    nc.sync.dma_start(out=wt, in_=w_d[i * P:(i + 1) * P, :])
                    # round(W/s_w) + MAGIC   (clip is a no-op for W by
                    # construction of w_scale = max|row|/127)
                    wr = wstage.tile([P, H], dt.float32, tag="wr")
                    nc.scalar.activation(wr, wt, Act.Copy, bias=MAGIC,
                                         scale=invw_sb[:, i:i + 1])
                    qw = wstage.tile([P, H], dt.bfloat16, tag="qw")
                    nc.vector.tensor_scalar(out=qw, in0=wr, scalar1=-MAGIC,
                                            scalar2=None, op0=Alu.add)
                    for k in range(KT):
                        nc.sync.dma_start(
                            out=WqT[:, k, i * P:(i + 1) * P],
                            in_=qw[:, k * P:(k + 1) * P],
                            transpose=True,
                        )

                # ---- main loop over token tiles ----
                for m in range(MT):
                    xt = xstage.tile([P, H], dt.float32, tag="xt")
                    nc.sync.dma_start(out=xt, in_=x_d[m * P:(m + 1) * P, :])
                    # t1 = x/s_a + MAGIC  (rounded to integer by fp32 arithmetic)
                    t1 = xstage.tile([P, H], dt.float32, tag="t1")
                    nc.scalar.activation(t1, xt, Act.Copy, bias=MAGIC, scale=inva_sb)
                    # clip in the magic-shifted domain
                    t2 = xstage.tile([P, H], dt.float32, tag="t2")
                    nc.vector.tensor_scalar(out=t2, in0=t1,
                                            scalar1=MAGIC - 128.0,
                                            scalar2=MAGIC + 127.0,
                                            op0=Alu.max, op1=Alu.min)
                    qx = qstage.tile([P, H], dt.bfloat16, tag="qx")
                    nc.vector.tensor_scalar(out=qx, in0=t2, scalar1=-MAGIC,
                                            scalar2=None, op0=Alu.add)
                    qxT = qtstage.tile([P, KT, P], dt.bfloat16, tag="qxT")
                    for k in range(KT):
                        nc.sync.dma_start(out=qxT[:, k, :],
                                          in_=qx[:, k * P:(k + 1) * P],
                                          transpose=True)

                    acc = psum.tile([P, NB, 512], dt.float32, tag="acc")
                    for n in range(NB):
                        for k in range(KT):
                            nc.tensor.matmul(
                                acc[:, n, :],
                                qxT[:, k, :],
                                WqT[:, k, n * 512:(n + 1) * 512],
                                start=(k == 0),
                                stop=(k == KT - 1),
                            )

                    # residual, cast to bf16 during the DMA (SWDGE)
                    rt = rstage.tile([P, H], dt.bfloat16, tag="rt")
                    nc.gpsimd.dma_start(out=rt, in_=r_d[m * P:(m + 1) * P, :])

                    # y = acc * (s_a*s_w[o]) + b[o]
                    ys = estage.tile([P, H], dt.float32, tag="ys")
                    nc.vector.tensor_mul(ys, acc[:, :, :].rearrange("p a b -> p (a b)"),
                                         scale_full)
                    yb = estage.tile([P, H], dt.bfloat16, tag="yb")
                    nc.vector.tensor_add(yb, ys, bias_full)
                    # h = bf16(y) + bf16(r)
                    ht = estage.tile([P, H], dt.bfloat16, tag="ht")
                    nc.vector.tensor_add(ht, yb, rt)

                    # layernorm stats
                    stats = vecs.tile([P, 2, 6], dt.float32, tag="stats")
                    nc.vector.bn_stats(stats[:, 0, :], ht[:, 0:512])
                    nc.vector.bn_stats(stats[:, 1, :], ht[:, 512:1024])
                    mv = vecs.tile([P, 2], dt.float32, tag="mv")
                    nc.vector.bn_aggr(mv, stats)
                    negmean = vecs.tile([P, 1], dt.float32, tag="negmean")
                    nc.vector.tensor_scalar(out=negmean, in0=mv[:, 0:1],
                                            scalar1=-1.0, scalar2=None, op0=Alu.mult)
                    stdv = vecs.tile([P, 1], dt.float32, tag="stdv")
                    nc.scalar.activation(stdv, mv[:, 1:2], Act.Sqrt,
                                         bias=eps_sb, scale=1.0)
                    rstd = vecs.tile([P, 1], dt.float32, tag="rstd")
                    nc.vector.reciprocal(rstd, stdv)

                    ot = ostage.tile([P, H], dt.float32, tag="ot")
                    nc.vector.tensor_scalar(out=ot, in0=ht, scalar1=negmean,
                                            scalar2=rstd, op0=Alu.add, op1=Alu.mult)
                    if apply_gamma:
                        og = ostage.tile([P, H], dt.float32, tag="og")
                        nc.vector.tensor_mul(og, ot, gamma_full)
                        ot = og
                    if apply_beta:
                        ob = ostage.tile([P, H], dt.float32, tag="ob")
                        nc.vector.tensor_add(ob, ot, beta_full)
                        ot = ob
                    nc.sync.dma_start(out=out_d[m * P:(m + 1) * P, :], in_=ot)

            if loop_reps:
                with tc.For_i(0, loop_reps, 1) as iv:
                    body(iv)
            else:
                body()

    nc.compile()
    return nc


@functools.lru_cache(maxsize=None)
def _get_program(apply_gamma: bool, apply_beta: bool, loop_reps: int = 0):
    return _build(apply_gamma, apply_beta, loop_reps)


def _make_in_maps(hidden_states, input_tensor, W, b, gamma, beta,
                  act_scale, w_scale, apply_gamma, apply_beta):
    f32 = np.float32
    W = np.ascontiguousarray(W, dtype=f32)
    scale_vec = (np.float32(act_scale) * w_scale).astype(f32)
    bias_vec = np.ascontiguousarray(b, dtype=f32)
    inv_act = np.full((P, 1), 1.0 / np.float32(act_scale), dtype=f32)
    inv_w = np.ascontiguousarray((1.0 / w_scale.astype(f32)).reshape(KT, P).T)
    in_maps = []
    for i in range(B):
        m = {
            "x": np.ascontiguousarray(hidden_states[i], dtype=f32),
            "r": np.ascontiguousarray(input_tensor[i], dtype=f32),
            "w": W,
            "scale_vec": scale_vec,
            "bias_vec": bias_vec,
            "inv_act": inv_act,
            "inv_w": inv_w,
        }
        if apply_gamma:
            m["gamma_vec"] = np.ascontiguousarray(gamma, dtype=f32)
        if apply_beta:
            m["beta_vec"] = np.ascontiguousarray(beta, dtype=f32)
        in_maps.append(m)
    return in_maps


def kernel(hidden_states, input_tensor, W, b, gamma, beta, act_scale, w_scale):
    apply_gamma = not np.all(gamma == 1.0)
    apply_beta = not np.all(beta == 0.0)
    nc = _get_program(apply_gamma, apply_beta, 0)
    in_maps = _make_in_maps(hidden_states, input_tensor, W, b, gamma, beta,
                            act_scale, w_scale, apply_gamma, apply_beta)
    res = run_bass_kernel_spmd(nc, in_maps, list(range(B)))
    out = np.stack([res.results[i]["out"] for i in range(B)], axis=0)
    return out.astype(np.float32)
